# revision 10
# baseline (speedup 1.0000x reference)
"""Trainium2 Bass kernel for nn_PlanNotesProjection.

Math (per batch b):
  own_f   = ownership[b].astype(f32)             # (K=32, S=4096)
  summed  = own_f @ emb[b]                       # (K, H=2048)
  counts  = clip(own_f.sum(-1), min=1)           # (K,)
  pooled  = summed / counts[:, None]
  proj    = pooled @ W + bias                    # (K, D=1024)
  out[b]  = LayerNorm(proj) * gamma + beta       # eps=1e-5

Structure: h-major, all matmul operands in bf16 (the harness gate is
rel_err < 2e-2; bf16 rounding contributes ~5e-3).  The host pre-swizzles
emb so that for each h-tile (128 columns of H) all 32 S-chunks are one
contiguous [128, xKB] DMA; DMAs carry 2 h-tiles (1 MB) for ~78% DMA
efficiency.  Pooling for h-tile h accumulates sumT_h[m, k] = sum_s
emb[s, h*128+m] * own[k, s] over the 32 S-chunks into a PSUM bank; as
soon as an h-tile finishes, its two projection matmuls (contraction
over H on partitions — no transpose) accumulate into the proj PSUM
banks while the next h-tile's DMA/pooling proceeds.  The 1/counts
scaling commutes past the projection matmul, so it is applied to proj.

Sharding: data-parallel over B across 8 cores (one batch per core);
W/b/gamma/beta replicated. Host swizzles make every device DMA fully
contiguous per partition:
  embP[p, (h*SC + c)*128 + j] = emb[c*128+p, h*128+j]   (bf16)
  ownP[p, c*K + k]            = own[k, c*128+p]          (bf16)
  wP[p, h*D + d]              = W[h*128+p, d]            (bf16)
"""

import sys
from contextlib import ExitStack

import ml_dtypes
import numpy as np

sys.path.insert(0, "/opt/trn_rl_repo")

B, K, S, H, D = 8, 32, 4096, 2048, 1024
LN_EPS = 1e-5
P = 128
SC = S // P    # 32 contraction chunks (S on partitions)
HC = H // P    # 16 h-tiles
DJ = D // 512  # 2 psum column tiles for projection
NHD = 2        # h-tiles per emb DMA (1 MB bf16 per DMA)

TRACE = False
LAST_RESULT = None
_NC = None

BF16 = ml_dtypes.bfloat16


def _prep_emb(emb_b):
    # (S, H) f32 -> (P, HC*SC*128) bf16 with embP[p, (h*SC+c)*128+j] = emb[c*128+p, h*128+j]
    return np.ascontiguousarray(
        emb_b.reshape(SC, P, HC, P).transpose(1, 2, 0, 3).reshape(P, HC * SC * P)
        .astype(BF16))


def _prep_own(own_b):
    # (K, S) bool -> (P, SC*K) bf16 with ownP[p, c*K+k] = own[k, c*128+p]
    return np.ascontiguousarray(
        own_b.T.astype(np.float32).reshape(SC, P, K).transpose(1, 0, 2).reshape(P, SC * K)
        .astype(BF16))


def _prep_w(wmat):
    # (H, D) f32 -> (P, HC*D) bf16 with wP[p, h*D+d] = W[h*128+p, d]
    return np.ascontiguousarray(
        wmat.reshape(HC, P, D).transpose(1, 0, 2).reshape(P, HC * D).astype(BF16))


def _build_nc(repeats=1):
    # repeats>1 unrolls the whole compute body (including DMAs) multiple
    # times in one NEFF; used by test.py to measure marginal per-iteration
    # HW time, cancelling host dispatch overhead. Grading uses repeats=1.
    import concourse.bass as bass
    import concourse.tile as tile
    from concourse import mybir
    from concourse.bacc import Bacc

    FP32 = mybir.dt.float32
    BF = mybir.dt.bfloat16

    # Bacc (not plain Bass): its finalize() runs the legalization passes
    # (move_matmul_waits_to_ldweights, generate_event_semaphores) that split
    # multi-semaphore waits — TRN2 TPB instructions carry at most one.
    nc = Bacc("TRN2", target_bir_lowering=False)
    embP = nc.declare_dram_parameter("embP", [P, HC * SC * P], BF, False)
    ownP = nc.declare_dram_parameter("ownP", [P, SC * K], BF, False)
    wP = nc.declare_dram_parameter("wP", [P, HC * D], BF, False)
    bvec = nc.declare_dram_parameter("bvec", [D], FP32, False)
    gamma = nc.declare_dram_parameter("gamma", [D], FP32, False)
    beta = nc.declare_dram_parameter("beta", [D], FP32, False)
    out = nc.declare_dram_parameter("out", [K, D], FP32, True)

    with ExitStack() as ctx:
        tc = ctx.enter_context(tile.TileContext(nc))

        # bufs=2 on own/w so the next repeat's loads don't WAR-wait on this
        # repeat's consumers (they head the SP HWDGE FIFO and would otherwise
        # head-of-line block the emb stream at the repeat boundary).
        own_pool = ctx.enter_context(tc.tile_pool(name="own", bufs=2))
        w_pool = ctx.enter_context(tc.tile_pool(name="w", bufs=2))
        emb_pool = ctx.enter_context(tc.tile_pool(name="emb", bufs=6))
        ones_pool = ctx.enter_context(tc.tile_pool(name="ones", bufs=1))
        eps_pool = ctx.enter_context(tc.tile_pool(name="eps", bufs=1))
        cnt_pool = ctx.enter_context(tc.tile_pool(name="cnt", bufs=1))
        st_pool = ctx.enter_context(tc.tile_pool(name="st", bufs=4))
        bc_pool = ctx.enter_context(tc.tile_pool(name="bc", bufs=1))
        x_pool = ctx.enter_context(tc.tile_pool(name="x", bufs=1))
        stats_pool = ctx.enter_context(tc.tile_pool(name="stats", bufs=1))
        mv_pool = ctx.enter_context(tc.tile_pool(name="mv", bufs=1))

        psum_sum = ctx.enter_context(tc.tile_pool(name="psum_sum", bufs=2, space="PSUM"))
        psum_proj = ctx.enter_context(tc.tile_pool(name="psum_proj", bufs=2, space="PSUM"))
        psum_cnt = ctx.enter_context(tc.tile_pool(name="psum_cnt", bufs=1, space="PSUM"))

        def body():
            own_sb = own_pool.tile([P, SC, K], BF)
            nc.sync.dma_start(own_sb[:], ownP[:, :])

            # whole W resident in SBUF (32 KB/partition bf16), one 4 MB DMA
            w_sb = w_pool.tile([P, HC, D], BF)
            nc.sync.dma_start(w_sb[:], wP[:, :])

            ones = ones_pool.tile([P, 1], BF)
            nc.vector.memset(ones[:], 1.0)
            eps = eps_pool.tile([K, 1], FP32)
            nc.vector.memset(eps[:], LN_EPS)

            def bcast(vec):
                t = bc_pool.tile([K, D], FP32, name=f"bc_{vec.name}")
                ap = vec[:]
                bc_ap = bass.AP(tensor=ap.tensor, offset=ap.offset, ap=[[0, K]] + list(ap.ap))
                nc.gpsimd.dma_start(out=t[:], in_=bc_ap)
                return t

            bias_bc = bcast(bvec)
            gam_bc = bcast(gamma)
            bet_bc = bcast(beta)

            # counts[k] = sum_s own[k, s]  (0/1 values, f32 PSUM accum: exact)
            cnt_ps = psum_cnt.tile([K, 1], FP32)
            for c in range(SC):
                nc.tensor.matmul(cnt_ps[:], own_sb[:, c, :], ones[:],
                                 start=(c == 0), stop=(c == SC - 1))
            cnt_sb = cnt_pool.tile([K, 1], FP32)
            nc.vector.tensor_scalar_max(out=cnt_sb[:], in0=cnt_ps[:], scalar1=1.0)
            inv_sb = cnt_pool.tile([K, 1], FP32)
            nc.vector.reciprocal(out=inv_sb[:], in_=cnt_sb[:])

            proj_ps = [psum_proj.tile([K, 512], FP32, name=f"proj_ps{jj}") for jj in range(DJ)]

            for hh in range(HC // NHD):
                et = emb_pool.tile([P, NHD * SC, P], BF)
                nc.sync.dma_start(et[:], embP[:, hh * NHD * SC * P:(hh + 1) * NHD * SC * P])
                for hl in range(NHD):
                    h = hh * NHD + hl
                    # Padded to 512 cols = 2KB = one full bank, so each
                    # ping-pong buf owns its bank and start=True can't touch
                    # a neighbour.
                    st_ps = psum_sum.tile([P, 512], FP32)
                    for c in range(SC):
                        nc.tensor.matmul(st_ps[:, 0:K], et[:, hl * SC + c, :],
                                         own_sb[:, c, :],
                                         start=(c == 0), stop=(c == SC - 1))
                    st_sb = st_pool.tile([P, K], BF)
                    nc.scalar.copy(out=st_sb[:], in_=st_ps[:, 0:K])
                    for jj in range(DJ):
                        nc.tensor.matmul(proj_ps[jj][:], st_sb[:],
                                         w_sb[:, h, jj * 512:(jj + 1) * 512],
                                         start=(h == 0), stop=(h == HC - 1))

            # --- epilogue: x = proj_raw/counts + bias; LayerNorm; *gamma + beta ---
            x = x_pool.tile([K, D], FP32)
            for jj in range(DJ):
                nc.vector.scalar_tensor_tensor(
                    out=x[:, jj * 512:(jj + 1) * 512], in0=proj_ps[jj][:],
                    scalar=inv_sb[:], in1=bias_bc[:, jj * 512:(jj + 1) * 512],
                    op0=mybir.AluOpType.mult, op1=mybir.AluOpType.add,
                )

            stats = stats_pool.tile([K, DJ, nc.vector.BN_STATS_DIM], FP32)
            for g in range(DJ):
                nc.vector.bn_stats(out=stats[:, g, :], in_=x[:, g * 512:(g + 1) * 512])
            mv = mv_pool.tile([K, nc.vector.BN_AGGR_DIM], FP32)
            nc.vector.bn_aggr(out=mv[:], in_=stats[:])
            nc.scalar.activation(
                out=mv[:, 1:2], in_=mv[:, 1:2],
                func=mybir.ActivationFunctionType.Sqrt, bias=eps[:], scale=1.0, alpha=0.0,
            )
            nc.vector.reciprocal(out=mv[:, 1:2], in_=mv[:, 1:2])
            # (x - mu) * gamma, then * rstd + beta  (two fused STT ops)
            normed = x_pool.tile([K, D], FP32)
            nc.vector.scalar_tensor_tensor(
                out=normed[:], in0=x[:], scalar=mv[:, 0:1], in1=gam_bc[:],
                op0=mybir.AluOpType.subtract, op1=mybir.AluOpType.mult,
            )
            outt = x_pool.tile([K, D], FP32)
            nc.vector.scalar_tensor_tensor(
                out=outt[:], in0=normed[:], scalar=mv[:, 1:2], in1=bet_bc[:],
                op0=mybir.AluOpType.mult, op1=mybir.AluOpType.add,
            )
            # out DMA on the ACT HWDGE ring: keeps the SP ring (emb stream)
            # free of the epilogue dependency chain.
            nc.scalar.dma_start(out[:, :], outt[:])

        for _ in range(repeats):
            body()

    nc.finalize()
    return nc


def kernel(**inputs: np.ndarray) -> np.ndarray:
    global _NC, LAST_RESULT
    from concourse.bass_utils import run_bass_kernel_spmd

    emb = np.asarray(inputs["plan_embeddings"], dtype=np.float32)
    own = np.asarray(inputs["ownership"])
    wmat = np.ascontiguousarray(np.asarray(inputs["W"], dtype=np.float32))
    bv = np.ascontiguousarray(np.asarray(inputs["b"], dtype=np.float32))
    ga = np.ascontiguousarray(np.asarray(inputs["gamma"], dtype=np.float32))
    be = np.ascontiguousarray(np.asarray(inputs["beta"], dtype=np.float32))

    if _NC is None:
        _NC = _build_nc()

    wP = _prep_w(wmat)
    in_maps = []
    for i in range(B):
        in_maps.append({
            "embP": _prep_emb(emb[i]),
            "ownP": _prep_own(own[i]),
            "wP": wP,
            "bvec": bv,
            "gamma": ga,
            "beta": be,
        })
    res = run_bass_kernel_spmd(_NC, in_maps, core_ids=list(range(B)), trace=TRACE)
    LAST_RESULT = res
    return np.stack([np.asarray(res.results[i]["out"]) for i in range(B)], axis=0).astype(np.float32)
